# revision 11
# baseline (speedup 1.0000x reference)
"""Trainium2 Bass kernel: multi-head attention block (B=4, N=2048, C=1024, H=16).

Sharding: 8 cores = (batch b in 0..3) x (head-group hg in 0..1, 8 heads each).
Each core computes qkv for its heads, full attention for its heads over its
batch, and a partial projection (its 512 rows of W_proj). Host sums the two
partials per batch and adds b_proj.

v3 design. The baseline was jointly ScalarE(exp)-bound (284us busy) and
TensorE-bound (359us busy of 394us), with ~70us of ScalarE idle during
qkv-production bursts.  Keys:
  - exp SPLIT between ScalarE (exact exp, 10/16 k-chunks) and VectorE
    (Schraudolph bit-trick exp: one tensor_scalar mult+add -> int16,
    bitcast bf16; HW round-to-nearest, ~2% rms err, 6/16 chunks).
    A -4ln2 shift (softmax-invariant) keeps both on one scale.
  - all matmuls bf16 (fp8 variants measured 3-7e-2 rel err - too lossy):
    S row-paired (two heads at PE row groups 0/64, K=64 each), O with the
    ones-column denominator trick (M=65), qkv/proj full 128x128.
  - st tiles are [128,512] per (chunk, head) so 4 PSUM banks double-buffer
    S two chunks ahead; matmuls are emitted in uniform shape/mode runs so
    LDWEIGHTS pipelines (mode switches cost ~100ns each).
  - qk/v/proj production is spread inside the attention loops as PE filler
    so ScalarE/VectorE never starve.
"""

import os
from collections import deque
from contextlib import ExitStack

import numpy as np
import ml_dtypes

import concourse.bass as bass
import concourse.tile as tile
from concourse import bacc, mybir
from concourse.bass import ds, ts
from concourse.bass_utils import run_bass_kernel_spmd

try:  # without the NTFF hook module, a stray BASS_TRACE=1 would crash the run
    from antenv.axon_hooks import get_axon_ntff_profile_hook  # noqa: F401
except ImportError:
    os.environ.setdefault("BASS_NEVER_TRACE", "1")

BF16 = mybir.dt.bfloat16
F32 = mybir.dt.float32
I16 = mybir.dt.int16
NP_BF16 = ml_dtypes.bfloat16

B, N, C = 4, 2048, 1024
H, D = 16, 64
HPC = 8            # heads per core
CD = HPC * D       # 512 local qkv dims per core
E = D + 1          # 65: 64 v dims + ones column

LN2 = float(np.log(2.0))
LOG2E = float(np.log2(np.e))
SHIFT = 4.0 * LN2          # scales all softmax terms by 2^-4 (invariant)
A16 = (2.0**7) * LOG2E / 8.0
B16 = 127.0 * 2**7 - (2.0**7) * LOG2E * SHIFT - 5.5

# per-block chunk engine pattern (16 k-chunks): A = ScalarE exact exp,
# D = VectorE Schraudolph.  10 A + 6 D balances both engines under PE.
PATTERN = "ADAADA" * 2 + "ADAA"

LAST_RESULTS = None  # stash for test harness (exec_time_ns, trace paths)


def _build_program(taps=False):
    nc = bacc.Bacc("TRN2", target_bir_lowering=False, debug=False)

    xT_d = nc.dram_tensor("xT", [C, N], BF16, kind="ExternalInput").ap()
    wqk_d = nc.dram_tensor("wqk", [C, 2 * CD], BF16, kind="ExternalInput").ap()
    wv_d = nc.dram_tensor("wv", [C, CD], BF16, kind="ExternalInput").ap()
    bqk_d = nc.dram_tensor("bqk", [128, 8], F32, kind="ExternalInput").ap()
    bv_d = nc.dram_tensor("bv", [1, CD], BF16, kind="ExternalInput").ap()
    wp_d = nc.dram_tensor("wp", [CD, C], BF16, kind="ExternalInput").ap()
    out_d = nc.dram_tensor("out", [N, C], F32, kind="ExternalOutput").ap()
    if taps:
        tap_qkT = nc.dram_tensor("tap_qkT", [128, 8, N], BF16, kind="ExternalOutput").ap()
        tap_v = nc.dram_tensor("tap_v", [128, 16 * HPC * E], BF16, kind="ExternalOutput").ap()
        tap_otc = nc.dram_tensor("tap_otc", [E, 512], F32, kind="ExternalOutput").ap()
        tap_oT = nc.dram_tensor("tap_oT", [128, 4, N], BF16, kind="ExternalOutput").ap()

    with tile.TileContext(nc) as tc, ExitStack() as ctx:
        singles = ctx.enter_context(tc.tile_pool(name="singles", bufs=1))
        st_pool = ctx.enter_context(tc.tile_pool(name="stp", bufs=4, space="PSUM"))
        ot_pool = ctx.enter_context(tc.tile_pool(name="ot", bufs=2, space="PSUM"))
        ps_pool = ctx.enter_context(tc.tile_pool(name="ps", bufs=2, space="PSUM"))
        ex_pool = ctx.enter_context(tc.tile_pool(name="ex", bufs=8))
        misc = ctx.enter_context(tc.tile_pool(name="misc", bufs=4))
        ob_pool = ctx.enter_context(tc.tile_pool(name="ob", bufs=3))

        # Persistent SBUF tensors, chunk-major: [partition, chunk, free].
        xT_sb = singles.tile([128, 8, N], BF16)        # x^T   [c, token]
        wqk_sb = singles.tile([128, 8, 2 * CD], BF16)  # W_qk  [c, m]
        wv_sb = singles.tile([128, 8, CD], BF16)       # W_v   [c, n]
        bqk_sb = singles.tile([128, 8], F32)
        bv_sb = singles.tile([1, CD], BF16)
        ones_sb = singles.tile([1, 128], BF16)
        wp_sb = singles.tile([128, 4, C], BF16)        # W_proj [hd, n]
        qkT_sb = singles.tile([128, 8, N], BF16)       # chunks 0..3 = q, 4..7 = k
        v_sb = singles.tile([128, 16, HPC * E], BF16)  # [tok, tchunk, h*(64+1)]
        oT_sb = singles.tile([128, 4, N], BF16)        # o^T, proj lhsT layout
        nbias_sb = singles.tile([128, 1], F32)         # -SHIFT for ACT exp

        nc.sync.dma_start(wv_sb, wv_d.rearrange("(c p) m -> p c m", p=128))
        nc.sync.dma_start(bv_sb, bv_d)
        for kc in range(8):
            nc.sync.dma_start(xT_sb[:, kc], xT_d.rearrange("(c p) t -> p c t", p=128)[:, kc])
        nc.sync.dma_start(wqk_sb, wqk_d.rearrange("(c p) m -> p c m", p=128))
        nc.sync.dma_start(bqk_sb, bqk_d)
        nc.sync.dma_start(wp_sb, wp_d.rearrange("(c p) n -> p c n", p=128))
        nc.vector.memset(ones_sb, 1.0)
        nc.vector.memset(nbias_sb, -SHIFT)
        nc.vector.memset(v_sb.rearrange("p t (h e) -> p t h e", e=E)[:, :, :, D : D + 1], 1.0)

        # ---- filler thunks ----------------------------------------------

        def v_chunk_thunks(t, quad):
            """v for token chunk t, head quad (4 heads): 8 matmuls + evac."""
            hold = {}
            n0 = quad * 256

            def mk(kc, t=t, hold=hold):
                def f():
                    if kc == 0:
                        hold["ps"] = ps_pool.tile([128, 256], F32, tag="fil", name="filps")
                    nc.tensor.matmul(
                        hold["ps"],
                        xT_sb[:, kc, ts(t, 128)],
                        wv_sb[:, kc, ds(n0, 256)],
                        start=(kc == 0),
                        stop=False,
                    )
                return f

            def bias_mm(hold=hold, n0=n0):
                nc.tensor.matmul(
                    hold["ps"], ones_sb, bv_sb[:, ds(n0, 256)], start=False, stop=True
                )

            def evac(t=t, quad=quad, hold=hold):
                dst = v_sb.rearrange("p t (h e) -> p t h e", e=E)[
                    :, t, 4 * quad : 4 * quad + 4, 0:D
                ]
                nc.vector.tensor_copy(
                    dst, hold["ps"].rearrange("p (h d) -> p h d", d=D)
                )

            return [mk(kc) for kc in range(8)] + [bias_mm, evac]

        def qk_chunk_thunks(m, quarters=range(4)):
            """qk dims chunk m (128 dims) for the given 512-token quarters."""
            out = []
            for i4q in quarters:
                hold = {}

                def mk(kc, m=m, i4q=i4q, hold=hold):
                    def f():
                        if kc == 0:
                            hold["ps"] = ps_pool.tile([128, 512], F32, tag="fil", name="filps")
                        nc.tensor.matmul(
                            hold["ps"],
                            wqk_sb[:, kc, ts(m, 128)],
                            xT_sb[:, kc, ds(i4q * 512, 512)],
                            start=(kc == 0),
                            stop=(kc == 7),
                        )
                    return f

                def evac(m=m, i4q=i4q, hold=hold):
                    nc.vector.tensor_scalar_add(
                        qkT_sb[:, m, ds(i4q * 512, 512)],
                        hold["ps"],
                        bqk_sb[:, ds(m, 1)],
                    )

                out += [mk(kc) for kc in range(8)] + [evac]
            return out

        def proj_thunks(t):
            """output projection for token chunk t (both 512-col halves)."""
            out = []
            for nh in range(2):
                hold = {}

                def mk(hc, t=t, nh=nh, hold=hold):
                    def f():
                        if hc == 0:
                            hold["ps"] = ps_pool.tile([128, 512], F32, tag="fil", name="filps")
                        nc.tensor.matmul(
                            hold["ps"],
                            oT_sb[:, hc, ts(t, 128)],
                            wp_sb[:, hc, ds(nh * 512, 512)],
                            start=(hc == 0),
                            stop=(hc == 3),
                        )
                    return f

                def evac(t=t, nh=nh, hold=hold):
                    ob = ob_pool.tile([128, 512], F32)
                    nc.vector.tensor_copy(ob, hold["ps"])
                    nc.sync.dma_start(out_d[ts(t, 128), ds(nh * 512, 512)], ob)

                out += [mk(hc) for hc in range(4)] + [evac]
            return out

        # ---- attention ---------------------------------------------------

        def evac_ot(ot, hp, p, i0, tap=False):
            otc = misc.tile([E, 512], F32, tag="otc")
            nc.vector.tensor_copy(otc, ot)
            if taps and tap:
                nc.sync.dma_start(tap_otc, otc)
            s_t = misc.tile([128, 4], F32, tag="sct")
            nc.sync.dma_start(s_t, otc[D : D + 1])
            r_t = misc.tile([128, 4], F32, tag="rct")
            nc.vector.reciprocal(r_t, s_t)
            rec0 = misc.tile([1, 512], F32, tag="rec0")
            nc.sync.dma_start(rec0, r_t)
            recb = misc.tile([D, 512], F32, tag="recb")
            nc.gpsimd.partition_broadcast(recb, rec0)
            nc.vector.tensor_mul(oT_sb[hp : hp + D, p, ds(i0, 512)], otc[0:D], recb)

        vv = v_sb.rearrange("p t (h e) -> p t h e", e=E)

        # upfront: k for pair 0 (all tokens), q for pair 0 (quarter 0),
        # v head-quad 0 (all chunks).  ~23us of PE before ScalarE engages.
        for th in qk_chunk_thunks(4):
            th()
        for th in qk_chunk_thunks(0, quarters=[0]):
            th()
        for t in range(16):
            for th in v_chunk_thunks(t, 0):
                th()

        for p in range(4):
            qA, qB = qkT_sb[0:64, p], qkT_sb[64:128, p]
            kA, kB = qkT_sb[0:64, 4 + p], qkT_sb[64:128, 4 + p]
            for i4 in range(4):
                i0 = i4 * 512

                fillers = deque()
                mq = p + 1  # next head-pair's qk chunks
                fmap = {
                    (0, 0): lambda: qk_chunk_thunks(0, [1, 2, 3]),
                    (0, 1): lambda: qk_chunk_thunks(4 + mq, [0, 1]),
                    (0, 2): lambda: qk_chunk_thunks(4 + mq, [2, 3])
                    + qk_chunk_thunks(mq, [0]),
                    (0, 3): lambda: qk_chunk_thunks(mq, [1, 2, 3]),
                    (1, 0): lambda: qk_chunk_thunks(4 + mq, [0, 1])
                    + v_chunk_thunks(0, 1) + v_chunk_thunks(1, 1),
                    (1, 1): lambda: qk_chunk_thunks(4 + mq, [2, 3])
                    + v_chunk_thunks(2, 1) + v_chunk_thunks(3, 1),
                    (1, 2): lambda: qk_chunk_thunks(mq, [0, 1])
                    + [t for i in range(4, 10) for t in v_chunk_thunks(i, 1)],
                    (1, 3): lambda: qk_chunk_thunks(mq, [2, 3])
                    + [t for i in range(10, 16) for t in v_chunk_thunks(i, 1)],
                    (2, 0): lambda: qk_chunk_thunks(4 + mq, [0, 1]),
                    (2, 1): lambda: qk_chunk_thunks(4 + mq, [2, 3]),
                    (2, 2): lambda: qk_chunk_thunks(mq, [0, 1]),
                    (2, 3): lambda: qk_chunk_thunks(mq, [2, 3]),
                    (3, 1): lambda: [t for i in range(0, 4) for t in proj_thunks(i)],
                    (3, 2): lambda: [t for i in range(4, 8) for t in proj_thunks(i)],
                    (3, 3): lambda: [t for i in range(8, 12) for t in proj_thunks(i)],
                }
                if (p, i4) in fmap:
                    fillers.extend(fmap[(p, i4)]())

                per_step = (len(fillers) + 7) // 8 if fillers else 0

                otA = ot_pool.tile([E, 512], F32, tag="ot")
                otB = ot_pool.tile([E, 512], F32, tag="ot")
                pend = deque()  # (c, exA, exB) awaiting O emission

                def emit_o_group():
                    # uniform run of up to 4 O matmuls (2 chunks x 2 heads)
                    items = list(pend)
                    pend.clear()
                    for h, ot in ((0, otA), (1, otB)):
                        for c, exA, exB in items:
                            ex = exA if h == 0 else exB
                            nc.tensor.matmul(
                                ot,
                                vv[:, c, 2 * p + h],
                                ex,
                                start=(c == 0),
                                stop=(c == 15),
                            )

                for g in range(8):  # groups of 2 chunks
                    exs = []
                    for c in (2 * g, 2 * g + 1):
                        stA = st_pool.tile([128, 512], F32, tag="st", name="stA")
                        stB = st_pool.tile([128, 512], F32, tag="st", name="stB")
                        nc.tensor.matmul(
                            stA, kA[:, ts(c, 128)], qA[:, ds(i0, 512)],
                            start=True, stop=True,
                        )
                        nc.tensor.matmul(
                            stB, kB[:, ts(c, 128)], qB[:, ds(i0, 512)],
                            start=True, stop=True,
                        )
                        exs.append((c, stA, stB))
                    for c, stA, stB in exs:
                        eng = PATTERN[c]
                        exA = ex_pool.tile([128, 512], BF16, tag="ex", name="exA")
                        exB = ex_pool.tile([128, 512], BF16, tag="ex", name="exB")
                        for st, ex in ((stA, exA), (stB, exB)):
                            if eng == "A":
                                nc.scalar.activation(
                                    ex, st, mybir.ActivationFunctionType.Exp,
                                    scale=1.0 / 8.0, bias=nbias_sb,
                                )
                            else:
                                nc.vector.tensor_scalar(
                                    ex.bitcast(I16), st, A16, B16,
                                    mybir.AluOpType.mult, mybir.AluOpType.add,
                                )
                        pend.append((c, exA, exB))
                    for _ in range(per_step):
                        if fillers:
                            fillers.popleft()()
                    if g >= 1:
                        emit_o_group()
                emit_o_group()
                while fillers:
                    fillers.popleft()()
                evac_ot(otA, 0, p, i0, tap=(p == 0 and i4 == 0))
                evac_ot(otB, 64, p, i0)

        if taps:
            nc.sync.dma_start(tap_qkT, qkT_sb)
            nc.sync.dma_start(tap_v, v_sb.rearrange("p t e -> p (t e)"))
            nc.sync.dma_start(tap_oT, oT_sb)

        # tail: last token quarter of the projection
        for t in range(12, 16):
            for th in proj_thunks(t):
                th()

    nc.compile()
    return nc


_PROGRAM = None


def kernel(x, W_qkv, b_qkv, W_proj, b_proj):
    global _PROGRAM, LAST_RESULTS
    x = np.asarray(x, dtype=np.float32)
    W_qkv = np.asarray(W_qkv, dtype=np.float32)
    b_qkv = np.asarray(b_qkv, dtype=np.float32)
    W_proj = np.asarray(W_proj, dtype=np.float32)
    b_proj = np.asarray(b_proj, dtype=np.float32)

    if _PROGRAM is None:
        _PROGRAM = _build_program()
    nc = _PROGRAM

    in_maps = []
    for core in range(8):
        b, hg = core // 2, core % 2
        h0 = hg * HPC
        sl = slice(h0 * D, h0 * D + CD)
        wq = W_qkv[:, 0 * C :][:, sl]
        wk = W_qkv[:, 1 * C :][:, sl]
        wv = W_qkv[:, 2 * C :][:, sl]
        bq = b_qkv[0 * C :][sl]
        bk = b_qkv[1 * C :][sl]
        bv = b_qkv[2 * C :][sl]
        in_maps.append(
            {
                "xT": np.ascontiguousarray(x[b].T).astype(NP_BF16),
                "wqk": np.concatenate([wq, wk], axis=1).astype(NP_BF16),
                "wv": np.ascontiguousarray(wv).astype(NP_BF16),
                "bqk": np.concatenate([bq, bk]).reshape(8, 128).T.astype(np.float32).copy(),
                "bv": bv.reshape(1, CD).astype(NP_BF16),
                "wp": np.ascontiguousarray(W_proj[sl, :]).astype(NP_BF16),
            }
        )

    res = run_bass_kernel_spmd(nc, in_maps, list(range(8)))
    LAST_RESULTS = res
    out = np.empty((B, N, C), dtype=np.float32)
    for b in range(B):
        out[b] = (
            res.results[2 * b]["out"].astype(np.float32)
            + res.results[2 * b + 1]["out"].astype(np.float32)
            + b_proj[None, :]
        )
    return out
